# revision 57
# baseline (speedup 1.0000x reference)
"""Trainium2 Bass kernel for dense multi-head causal self-attention.

Problem: hidden_states [2, 2048, 2048], w_qkv [6144, 2048], w_out [2048, 2048],
16 heads x 128 head_dim, causal softmax attention + out projection.

Sharding: tensor-parallel over heads. Each of the 8 cores computes 2 heads
(qkv projection for its 768 w_qkv rows, causal attention, partial
out-projection against its 256 w_out columns); the host sums the 8 partial
bf16 outputs in f32 (the "all-reduce").

Schedule: attention alone is ScalarE-bound (exp costs ~1.4x its PE work)
while the projections are PE-bound, so the kernel runs three sections that
keep every engine fed continuously:
  A. full qkv projection for batch 0's four token blocks, then v-only
     projection for batch 1's blocks (q,k transposed [d, tok] via w-stationary
     matmuls; v natural [tok, d] via x-stationary matmuls; one 2MB x DMA per
     token block, prefetched up to 3 deep for the DMA-hungry v-only stretch)
  B. batch 0's attention with batch 1's q/k projection woven INTO the
     attention k-tile loops (a generator emits one projection ko-step per
     consumed score tile; per-row top-ups keep the proj on schedule)
  C. batch 1's attention with out-projection m-tiles of completed token
     blocks woven in the same way (up to 2 per consumed tile), then a drain
     that rotates across every freed PSUM bank

Attention per block: scores transposed [k, q], exp on ScalarE (bf16 probs),
causal mask multiply on GpSimd pipelined one k-tile behind; probs compressed
across k-tiles on the DVE (s_t += pr, first two tiles folded into one add)
so the softmax denominator costs one all-ones matmul per block, landing
broadcast on every partition; normalize = reciprocal_approx_fast + one fused
psum*recip multiply on the DVE.  Each block's tail (last P@V, denominator,
normalize) is deferred past the next block's first three scores matmuls.
Out-projection staging copies split ScalarE/DVE 7:9, quad-batched output
DMAs.
"""

import sys

sys.path.insert(0, "/opt/trn_rl_repo")

import numpy as np

B, T, H, NH, HD = 2, 2048, 2048, 16, 128
TOK = B * T  # 4096
P = 128
NCORES = 8
HPC = NH // NCORES  # heads per core = 2
SCALE = 1.0 / float(np.sqrt(HD))
QB = 512  # query block
KT = H // P  # 16 contraction tiles for qkv
NTB = TOK // QB  # 8 token blocks
NQK = 2 * HPC  # q,k output row-tiles per core
VW = HPC * HD  # v width (both heads) = 256
NM = H // P  # out-projection row tiles = 16

_CACHE = {}


def _build():
    import concourse.bacc as bacc
    import concourse.mybir as mybir
    import concourse.tile as tile

    dt = mybir.dt
    f32 = dt.float32
    bf16 = dt.bfloat16
    AF = mybir.ActivationFunctionType
    ALU = mybir.AluOpType

    nc = bacc.Bacc(None, target_bir_lowering=False, debug=True)
    # token-block-major layouts: one contiguous run per partition per DMA
    xT = nc.dram_tensor("xT", [P, NTB, KT, QB], bf16, kind="ExternalInput")
    wqkvT = nc.dram_tensor("wqkvT", [P, KT, 6 * P], bf16, kind="ExternalInput")
    woutT = nc.dram_tensor("woutT", [P, HPC, H], bf16, kind="ExternalInput")
    tri = nc.dram_tensor("tri", [P, P], bf16, kind="ExternalInput")
    ones = nc.dram_tensor("ones", [P, P], bf16, kind="ExternalInput")
    outT = nc.dram_tensor("outT", [P, NTB, NM, QB], bf16, kind="ExternalOutput")

    with tile.TileContext(nc) as tc:
        with tc.tile_pool(name="const", bufs=1) as constp, \
             tc.tile_pool(name="qk", bufs=1) as qkp:
            tri_sb = constp.tile([P, P], bf16)
            ones_sb = constp.tile([P, P], bf16)

            qT_sb = qkp.tile([P, HPC, TOK], bf16)  # [d, h, tok]
            kT_sb = qkp.tile([P, HPC, TOK], bf16)  # [d, h, tok]
            v_sb = qkp.tile([P, TOK // P, VW], bf16)  # [tok%P, tok//P, h*HD+d]
            wo_sb = qkp.tile([P, HPC, H], bf16)  # long-lived: loads early

            # ---------- Section A: qkv proj for b0 + v proj for b1 ----------
            # q,k transposed: psum[o,tok] = w_slice.T @ xT ; v natural:
            # psum[tok,d] = xT_chunk.T @ w_vT.  b1's q/k projection is
            # deferred to section B where it weaves into b0's attention so
            # the ScalarE exp load spreads over the whole timeline.
            with tc.tile_pool(name="w1", bufs=1) as w1p, \
                 tc.tile_pool(name="x1", bufs=4) as x1p:
                w_sb = w1p.tile([P, KT, 6 * P], bf16)
                x_tiles = {}

                def load_x(tb, gen=""):
                    xt = x1p.tile([P, KT, QB], bf16, tag="x",
                                  name=f"x_{tb}{gen}")
                    nc.sync.dma_start(xt[:], xT[:, tb, :, :])
                    x_tiles[tb] = xt

                # tb0's x and the weights arrive in ko-consumption order on
                # two parallel issue streams (x on the sync sequencer, w and
                # constants on the scalar sequencer) so the PE is never
                # starved by serialized DMA issues
                xt0 = x1p.tile([P, KT, QB], bf16, tag="x", name="x_0")
                x_tiles[0] = xt0
                # w is split across BOTH hardware DMA rings (the sync ring
                # observably streams ~2x faster than the scalar ring at
                # startup), interleaved with x0's chunks in ko order so each
                # ko's x+w pair lands just ahead of the PE
                nc.scalar.dma_start(w_sb[:, 0:1, :], wqkvT[:, 0:1, :])
                nc.sync.dma_start(xt0[:, 0:1, :], xT[:, 0, 0:1, :])
                nc.sync.dma_start(w_sb[:, 1:3, :], wqkvT[:, 1:3, :])
                nc.scalar.dma_start(w_sb[:, 3:5, :], wqkvT[:, 3:5, :])
                nc.sync.dma_start(xt0[:, 1:3, :], xT[:, 0, 1:3, :])
                nc.sync.dma_start(w_sb[:, 5:8, :], wqkvT[:, 5:8, :])
                nc.sync.dma_start(xt0[:, 3:5, :], xT[:, 0, 3:5, :])
                nc.scalar.dma_start(w_sb[:, 8:11, :], wqkvT[:, 8:11, :])
                nc.sync.dma_start(xt0[:, 5:8, :], xT[:, 0, 5:8, :])
                nc.sync.dma_start(w_sb[:, 11:14, :], wqkvT[:, 11:14, :])
                nc.sync.dma_start(xt0[:, 8:11, :], xT[:, 0, 8:11, :])
                nc.scalar.dma_start(w_sb[:, 14:KT, :], wqkvT[:, 14:KT, :])
                nc.sync.dma_start(xt0[:, 11:14, :], xT[:, 0, 11:14, :])
                nc.sync.dma_start(xt0[:, 14:KT, :], xT[:, 0, 14:KT, :])
                nc.scalar.dma_start(tri_sb[:], tri[:])
                nc.scalar.dma_start(ones_sb[:], ones[:])
                nc.scalar.dma_start(wo_sb[:], woutT[:])

                def emit_v(psv, tb, x_t):
                    ps_v = [
                        psv.tile([P, VW], f32, tag=f"psv{c}",
                                 name=f"psv{c}_{tb}")
                        for c in range(4)
                    ]
                    for ko in range(KT):
                        xk = x_t[:, ko, :]
                        for c in range(4):
                            nc.tensor.matmul(
                                ps_v[c][:],
                                xk[:, c * P:(c + 1) * P],
                                w_sb[:, ko, NQK * P:],
                                start=(ko == 0),
                                stop=(ko == KT - 1),
                            )
                        yield ko
                    for c in range(4):
                        if tb == NTB - 1:
                            # the last block's v-psum banks are reused by
                            # section B's first proj matmuls: drain them on
                            # the idle DVE, not behind ScalarE's queue
                            nc.vector.tensor_copy(v_sb[:, tb * 4 + c, :],
                                                  ps_v[c][:])
                        else:
                            nc.scalar.copy(v_sb[:, tb * 4 + c, :], ps_v[c][:])

                def qk_dst(m, tb):
                    if m < HPC:
                        return qT_sb[:, m, tb * QB:(tb + 1) * QB]
                    return kT_sb[:, m - HPC, tb * QB:(tb + 1) * QB]

                loaded = [0]

                def load_upto(tb):
                    # the v-only stretch consumes x nearly as fast as one
                    # DMA stream delivers it, so run the prefetch 3 blocks
                    # deep (bufs=4) while the b0 stretch has DMA slack
                    while loaded[0] < min(tb, NTB - 1):
                        loaded[0] += 1
                        load_x(loaded[0])

                with tc.tile_pool(name="ps1", bufs=1, space="PSUM") as ps1, \
                     tc.tile_pool(name="psv", bufs=1, space="PSUM") as psv:
                    for tb in range(NTB // 2):  # b0: full qkv projection
                        # don't flood the startup window: tb0 prefetches one
                        # block, later blocks run the queue 3 deep for the
                        # DMA-hungry v-only stretch
                        load_upto(tb + (1 if tb == 0 else 3))
                        x_t = x_tiles.pop(tb)
                        ps_qk = [
                            ps1.tile([P, QB], f32, tag=f"psqk{m}",
                                     name=f"psqk{m}_{tb}")
                            for m in range(NQK)
                        ]
                        vgen = emit_v(psv, tb, x_t)
                        for ko in vgen:
                            for m in range(NQK):
                                nc.tensor.matmul(
                                    ps_qk[m][:],
                                    w_sb[:, ko, m * P:(m + 1) * P],
                                    x_t[:, ko, :],
                                    start=(ko == 0),
                                    stop=(ko == KT - 1),
                                )
                        for m in range(NQK):
                            nc.vector.tensor_copy(qk_dst(m, tb), ps_qk[m][:])
                    for tb in range(NTB // 2, NTB):  # b1: v only
                        load_upto(tb + 2)
                        if tb == NTB - 1:
                            # prime section B's first q/k x reload
                            load_x(NTB // 2, gen="r")
                        x_t = x_tiles.pop(tb)
                        for _ in emit_v(psv, tb, x_t):
                            pass

                # ---------- Sections B+C: attention with woven fillers ------
                with tc.tile_pool(name="attn", bufs=1) as attnp:
                    attn_sb = attnp.tile([P, HPC, TOK], bf16)  # [d, h, tok]

                    with tc.tile_pool(name="pr", bufs=8) as prp, \
                         tc.tile_pool(name="o3", bufs=6) as o3p, \
                         tc.tile_pool(name="nrm", bufs=2) as nrmp, \
                         tc.tile_pool(name="ps_sc", bufs=3,
                                      space="PSUM") as ps_sc, \
                         tc.tile_pool(name="ps_at", bufs=2,
                                      space="PSUM") as ps_at, \
                         tc.tile_pool(name="ps_sm", bufs=1,
                                      space="PSUM") as ps_sm:

                        # the tail of each block (last P@V matmul, denominator
                        # matmul, reciprocal, normalize) is deferred into the
                        # next block, after its first scores matmuls are
                        # queued, so the PE never waits on the last probs tile
                        deferred = [None]
                        # attention alone is ScalarE-bound (exp costs ~1.4x
                        # its PE work), so PE-heavy filler work — b1's q/k
                        # projection in section B, out-projection m-tiles in
                        # section C — is woven INTO the attention k-tile
                        # loops one piece per consumed tile, keeping every
                        # engine fed continuously
                        op_queue = []
                        op_grp = {}
                        ps3_holder = [None]

                        # staging split biased toward ScalarE: exp leaves it
                        # ~50% loaded while the DVE carries S-compression +
                        # normalize + its staging share
                        scalar_ms = {0, 2, 4, 6, 8, 10, 12}

                        drain_idx = [0]

                        def emit_op(tb, m, final=False, via_sm=False):
                            mg, mi = divmod(m, 4)
                            key = (tb, mg)
                            if key not in op_grp:
                                op_grp[key] = o3p.tile([P, 4, QB], bf16,
                                                       tag="osb",
                                                       name=f"osb_{tb}_{mg}")
                            o_q = op_grp[key]
                            if via_sm:
                                # widen the outproj psum ring: the
                                # denominator bank is free mid-block
                                ps = ps_sm.tile([P, QB], f32, tag="sums",
                                                name=f"out_{tb}_{m}")
                            elif final:
                                # attention done: rotate through every freed
                                # psum bank (ring of 8) so drain matmuls
                                # never wait on staging
                                pool, tag = (
                                    (ps3_holder[0], "out"), (ps_sc, "sc"),
                                    (ps_at, "attn"), (ps_sc, "sc"),
                                    (ps_sm, "sums"), (ps_sc, "sc"),
                                    (ps_at, "attn"), (ps3_holder[0], "out"),
                                )[drain_idx[0] % 8]
                                drain_idx[0] += 1
                                ps = pool.tile([P, QB], f32, tag=tag,
                                               name=f"out_{tb}_{m}")
                            else:
                                ps = ps3_holder[0].tile([P, QB], f32,
                                                        tag="out",
                                                        name=f"out_{tb}_{m}")
                            for ko in range(HPC):
                                nc.tensor.matmul(
                                    ps[:],
                                    wo_sb[:, ko, m * P:(m + 1) * P],
                                    attn_sb[:, ko, tb * QB:(tb + 1) * QB],
                                    start=(ko == 0),
                                    stop=(ko == HPC - 1),
                                )
                            if final and len(op_queue) < 3:
                                # final tiles: halve the staging latency on
                                # the exit-critical path by splitting each
                                # copy across both engines
                                nc.scalar.copy(o_q[:, mi, :QB // 2],
                                               ps[:, :QB // 2])
                                nc.vector.tensor_copy(o_q[:, mi, QB // 2:],
                                                      ps[:, QB // 2:])
                            elif final:
                                # drain: strict alternation for an even
                                # engine split
                                if drain_idx[0] % 2 == 0:
                                    nc.scalar.copy(o_q[:, mi, :], ps[:])
                                else:
                                    nc.vector.tensor_copy(o_q[:, mi, :],
                                                          ps[:])
                            elif m % 16 in scalar_ms:
                                nc.scalar.copy(o_q[:, mi, :], ps[:])
                            else:
                                nc.vector.tensor_copy(o_q[:, mi, :], ps[:])
                            if mi == 3:
                                del op_grp[key]
                                if final and not op_queue:
                                    # smallest writes at the very end,
                                    # issued on both sequencers so the
                                    # 565ns-per-issue cost doesn't
                                    # serialize the exit
                                    for hq in range(4):
                                        eng = (nc.sync if hq % 2 == 0
                                               else nc.scalar)
                                        eng.dma_start(
                                            outT[:, tb,
                                                 mg * 4 + hq:
                                                 mg * 4 + hq + 1, :],
                                            o_q[:, hq:hq + 1, :],
                                        )
                                else:
                                    nc.sync.dma_start(
                                        outT[:, tb, mg * 4:(mg + 1) * 4, :],
                                        o_q[:],
                                    )

                        op_ctr = [0]

                        def pop_op(kt, n_k):
                            for _ in range(min(2, len(op_queue))):
                                tb, m = op_queue.pop(0)
                                # every third tile borrows the sums bank,
                                # but never near block boundaries where the
                                # deferred denominator matmul claims it
                                use_sm = (2 <= kt <= n_k - 2
                                          and op_ctr[0] % 3 == 2)
                                op_ctr[0] += 1
                                emit_op(tb, m, via_sm=use_sm)

                        def flush_tail():
                            if deferred[0] is not None:
                                fin, done_tb = deferred[0]
                                deferred[0] = None
                                fin()
                                if done_tb is not None:
                                    op_queue.extend(
                                        (done_tb, m) for m in range(NM))

                        def attn_block(b, h, j, filler=None):
                            base = b * T
                            q_ap = qT_sb[:, h,
                                         base + j * QB: base + (j + 1) * QB]
                            n_k = (j + 1) * (QB // P)
                            attn_ps = ps_at.tile([P, QB], f32, tag="attn",
                                                 name=f"at_{b}_{h}_{j}")
                            s_t = nrmp.tile([P, QB], bf16, tag="S",
                                            name=f"S_{b}_{h}_{j}")

                            def koff(kt):
                                diag = kt - j * (QB // P)
                                return diag * P if diag >= 0 else 0

                            def emit_scores(kt):
                                off = koff(kt)
                                sc = ps_sc.tile([P, QB], f32, tag="sc",
                                                name=f"sc_{b}_{h}_{j}_{kt}")
                                nc.tensor.matmul(
                                    sc[:, off:],
                                    kT_sb[:, h,
                                          base + kt * P: base + (kt + 1) * P],
                                    q_ap[:, off:],
                                    start=True,
                                    stop=True,
                                )
                                return sc

                            prs = {}

                            def consume(kt):
                                # P@V matmul + S compression for a masked
                                # tile; runs one k-tile behind the exp/mask
                                # producers.  The first two tiles fold into
                                # one s_t = p0+p1 add; the final tile skips
                                # the DVE add entirely: it joins the
                                # denominator as a second sums-matmul
                                # accumulation call.
                                off = koff(kt)
                                pr = (prs[kt] if kt in (0, n_k - 1)
                                      else prs.pop(kt))
                                nc.tensor.matmul(
                                    attn_ps[:, off:],
                                    v_sb[:, b * (T // P) + kt,
                                         h * HD:(h + 1) * HD],
                                    pr[:, off:],
                                    start=(kt == 0),
                                    stop=(kt == n_k - 1),
                                )
                                if kt == 1:
                                    pr0 = prs.pop(0)
                                    if off > 0:
                                        nc.vector.tensor_copy(s_t[:, :off],
                                                              pr0[:, :off])
                                        nc.vector.tensor_tensor(
                                            s_t[:, off:], pr0[:, off:],
                                            pr[:, off:], ALU.add,
                                        )
                                    else:
                                        nc.vector.tensor_tensor(
                                            s_t[:], pr0[:], pr[:], ALU.add,
                                        )
                                elif 1 < kt < n_k - 1:
                                    nc.vector.tensor_tensor(
                                        s_t[:, off:], s_t[:, off:],
                                        pr[:, off:], ALU.add,
                                    )

                            sc_q = [emit_scores(0)]
                            if n_k > 1:
                                sc_q.append(emit_scores(1))
                            for kt in range(n_k):
                                off = koff(kt)
                                diag = kt - j * (QB // P)
                                pr = prp.tile([P, QB], bf16, tag="pr",
                                              name=f"pr_{b}_{h}_{j}_{kt}")
                                prs[kt] = pr
                                nc.scalar.activation(
                                    pr[:, off:], sc_q[kt][:, off:], AF.Exp,
                                    scale=SCALE,
                                )
                                if kt + 2 < n_k:
                                    sc_q.append(emit_scores(kt + 2))
                                if kt == 1:
                                    # flush the previous block's deferred
                                    # tail here: three scores matmuls of PE
                                    # runway now precede its last-probs wait
                                    flush_tail()
                                if diag >= 0:
                                    # j=0 blocks are all-diagonal and too
                                    # shallow to hide GpSimd mask latency;
                                    # the DVE is idle at their start and has
                                    # a faster sem path
                                    eng = nc.vector if j == 0 else nc.gpsimd
                                    eng.tensor_tensor(
                                        pr[:, off:off + P],
                                        pr[:, off:off + P],
                                        tri_sb[:], ALU.mult,
                                    )
                                if kt >= 1:
                                    # filler BEFORE the consume: its matmuls
                                    # depend on nothing recent, giving
                                    # exp/mask of tile kt-1 time to finish
                                    # before the P@V matmul needs them
                                    if filler is not None:
                                        filler(kt, n_k)
                                    consume(kt - 1)

                            def fin():
                                consume(n_k - 1)
                                last_off = koff(n_k - 1)
                                sums_ps = ps_sm.tile([P, QB], f32, tag="sums",
                                                     name=f"sm_{b}_{h}_{j}")
                                nc.tensor.matmul(
                                    sums_ps[:], ones_sb[:], s_t[:],
                                    start=True, stop=False,
                                )
                                nc.tensor.matmul(
                                    sums_ps[:, last_off:], ones_sb[:],
                                    prs.pop(n_k - 1)[:, last_off:],
                                    start=False, stop=True,
                                )
                                rcp = nrmp.tile([P, QB], f32, tag="rcp",
                                                name=f"rcp_{b}_{h}_{j}")
                                nc.vector.reciprocal_approx_fast(rcp[:],
                                                                 sums_ps[:])
                                nc.vector.tensor_tensor(
                                    attn_sb[:, h,
                                            base + j * QB:
                                            base + (j + 1) * QB],
                                    attn_ps[:], rcp[:], ALU.mult,
                                )

                            tb_done = (b * (T // QB) + j) \
                                if h == HPC - 1 else None
                            deferred[0] = (fin, tb_done)

                        # ------- Section B: b0 attention + b1 q/k proj ------
                        with tc.tile_pool(name="psB", bufs=1,
                                          space="PSUM") as psB:
                            def proj_b1():
                                for tb in range(NTB // 2, NTB):
                                    if tb + 1 < NTB:
                                        load_x(tb + 1, gen="r")
                                    x_t = x_tiles.pop(tb)
                                    for p in range(2):
                                        ps_qk = [
                                            psB.tile([P, QB], f32,
                                                     tag=f"qkB{i}",
                                                     name=f"qkB_{tb}_{p}{i}")
                                            for i in range(2)
                                        ]
                                        for ko in range(KT):
                                            for i in range(2):
                                                m = 2 * p + i
                                                nc.tensor.matmul(
                                                    ps_qk[i][:],
                                                    w_sb[:, ko,
                                                         m * P:(m + 1) * P],
                                                    x_t[:, ko, :],
                                                    start=(ko == 0),
                                                    stop=(ko == KT - 1),
                                                )
                                            yield
                                        for i in range(2):
                                            nc.vector.tensor_copy(
                                                qk_dst(2 * p + i, tb),
                                                ps_qk[i][:])
                                        # a woven consume between these
                                        # copies and the next pass's psum
                                        # reuse hides the copy latency
                                        yield

                            gen = proj_b1()
                            nsteps = [0]

                            def pop_proj(kt=0, n_k=0):
                                for _ in gen:
                                    nsteps[0] += 1
                                    return

                            def top_up(target):
                                while nsteps[0] < target:
                                    try:
                                        next(gen)
                                    except StopIteration:
                                        break
                                    nsteps[0] += 1

                            for j in range(T // QB):
                                for h in range(HPC):
                                    attn_block(0, h, j, filler=pop_proj)
                                # stay on schedule: tb 4+j's q/k (incl. its
                                # copies, emitted before the pass-end yield)
                                # must be done by the end of attn row j
                                top_up((j + 1) * 2 * (KT + 1))
                            for _ in gen:  # exhaust: emit tb7's q/k copies
                                pass

                        # ------- Section C: b1 attention + out-projection ---
                        with tc.tile_pool(name="ps3", bufs=2,
                                          space="PSUM") as ps3:
                            ps3_holder[0] = ps3
                            # descending j: the heavy rows run while the
                            # out-projection queue is rich, and the final
                            # (filler-starved) block is the smallest one
                            for j in range(T // QB - 1, -1, -1):
                                for h in range(HPC):
                                    attn_block(1, h, j, filler=pop_op)
                            flush_tail()
                            while op_queue:
                                emit_op(*op_queue.pop(0), final=True)
    nc.finalize()
    return nc


def _host_inputs(hidden_states, w_qkv, w_out):
    import ml_dtypes

    BF16 = np.dtype(ml_dtypes.bfloat16)
    x = np.asarray(hidden_states, dtype=np.float32).reshape(TOK, H)
    w_qkv = np.asarray(w_qkv, dtype=np.float32)
    w_out = np.asarray(w_out, dtype=np.float32)

    # [P, NTB, KT, QB]: xT4[p, tb, ko, q] = x.T[ko*P + p, tb*QB + q]
    xT4 = np.ascontiguousarray(
        x.T.reshape(KT, P, NTB, QB).transpose(1, 2, 0, 3)
    ).astype(BF16)

    # lower-triangle-inclusive mask for the diagonal 128x128 strip
    tri = (np.arange(P)[:, None] <= np.arange(P)[None, :]).astype(BF16)
    ones = np.ones((P, P), dtype=BF16)

    in_maps = []
    for c in range(NCORES):
        heads = [HPC * c + i for i in range(HPC)]
        rows = []
        for sec in range(2):  # q, k sections
            for hh in heads:
                rows.append(w_qkv[sec * H + hh * HD: sec * H + (hh + 1) * HD])
        for hh in heads:  # v section
            rows.append(w_qkv[2 * H + hh * HD: 2 * H + (hh + 1) * HD])
        w_slice = np.concatenate(rows, axis=0)  # [768, H]
        wqkvT3 = np.ascontiguousarray(
            w_slice.T.reshape(KT, P, 6 * P).transpose(1, 0, 2)
        ).astype(BF16)  # [P, KT, 768]
        cols = np.concatenate([np.arange(hh * HD, (hh + 1) * HD) for hh in heads])
        woutT3 = np.ascontiguousarray(
            w_out[:, cols].T.reshape(HPC, P, H).transpose(1, 0, 2)
        ).astype(BF16)  # [P, HPC, H]
        in_maps.append({
            "xT": xT4,
            "wqkvT": wqkvT3,
            "woutT": woutT3,
            "tri": tri,
            "ones": ones,
        })
    return in_maps


def _assemble(res):
    # outT is [P, NTB, NM, QB]; row m*P+p, col tb*QB+q of the logical
    # [H, TOK] partial lives at outT[p, tb, m, q]
    acc = res.results[0]["outT"].astype(np.float32)
    for c in range(1, NCORES):
        acc = acc + res.results[c]["outT"].astype(np.float32)
    full = acc.transpose(2, 0, 1, 3).reshape(H, TOK)  # [H, TOK]
    return np.ascontiguousarray(full.T).reshape(B, T, H)


def _run(in_maps, trace=False):
    from concourse.bass_utils import run_bass_kernel_spmd

    if "nc" not in _CACHE:
        _CACHE["nc"] = _build()
    return run_bass_kernel_spmd(
        _CACHE["nc"], in_maps, core_ids=list(range(NCORES)), trace=trace
    )


def kernel(hidden_states, w_qkv, w_out):
    in_maps = _host_inputs(hidden_states, w_qkv, w_out)
    res = _run(in_maps)
    return _assemble(res).astype(np.float32)

